# revision 33
# baseline (speedup 1.0000x reference)
"""Block-sparse DSD matmul  y = x @ W^T  on 8 TRN2 NeuronCores.

x: [2048, 4096] f32, W given as 2048 sparse 32x32 blocks at (rows, cols)
block coordinates in a 128x128 block grid. y: [2048, 4096] f32.

Strategy (batch-parallel SPMD, identical program on 8 cores):
  - Shard batch 8 ways (256 rows/core); the sparse structure is identical
    on every core so one SPMD program works with per-core x shards.
  - All tensors cast to bf16 on host: PE matmuls run 1-pass (fp32 ran
    LOW_HIGH 2-pass) and HBM traffic halves.  PSUM accumulates f32;
    y is written back bf16 and widened on host.
  - Compute y^T tiles on-chip: for block (r, c):
        y^T[32r:32r+32, :] += W_blk @ x^T[32c:32c+32, :]
    As a PE matmul: out = lhsT.T @ rhs with lhsT = W_blk^T (stationary,
    32x32), rhs = x^T chunk [32, 256].
  - 16-way 32x32 PE subarray tiling: lane a = c%4 picks the SBUF
    partition strip (and PE row group); row-blocks are packed 4 to a
    "group", strip b in the group picks the PSUM partition strip (PE col
    group).  Each lane accumulates into its own PSUM bank; the 4 lane
    banks fold via ACT (2 copies), DVE (2 adds) and Pool (final add +
    bf16 cast) so no single engine serializes the evacuation.
  - DMA: x in 4 chunks + w in 8 chunks (4 groups each) on the Sync HWDGE
    queue; y^T written per 4 groups on the Scalar HWDGE queue so output
    never queues behind input loads.  All lines >= 2 KB.
  - Host: pre-transposes x into partition-major per-core layout, packs
    transposed weight blocks into a lane-major array, assembles y.
"""

import numpy as np
import ml_dtypes

BF16 = ml_dtypes.bfloat16

# toggles used by test.py only; harness uses defaults
_RUN = {"trace": False, "trace_cores": [0], "last": None}

B, K, OUT, BLK, NNZ = 2048, 4096, 4096, 32, 2048
NCORES = 8
BC = B // NCORES          # 256 batch rows per core
NT = K // 128             # 32 x^T partition-tiles
NRB = OUT // BLK          # 128 row blocks
NG = NRB // 4             # 32 groups of 4 row blocks
GC = 4                    # groups per w-DMA chunk / y-DMA batch
NWC = NG // GC            # 8 w chunks
XCH = 4                   # x chunks


def _build_schedule(w, rows, cols):
    """Group assignment + per-(group, lane) slot schedule + packed weights."""
    cnt = np.bincount(rows, minlength=NRB)
    order = np.argsort(-cnt, kind="stable")
    rmap = np.empty((NG, 4), dtype=np.int64)
    for rank, r in enumerate(order):
        rnd, pos = rank // NG, rank % NG
        g = pos if rnd % 2 == 0 else NG - 1 - pos
        rmap[g, rnd] = r

    gb_of_row = {}
    for g in range(NG):
        for b in range(4):
            gb_of_row[int(rmap[g, b])] = (g, b)

    cells = [[[[] for _ in range(4)] for _ in range(4)] for _ in range(NG)]
    for n in range(NNZ):
        g, b = gb_of_row[int(rows[n])]
        cells[g][int(cols[n]) % 4][b].append(n)

    # prog[g][a] = list of slots (t, b, start, stop, wT[32,32]), sorted by
    # x-tile index t so matmuls become eligible as x chunks stream in.
    prog = []
    for g in range(NG):
        lanes = []
        for a in range(4):
            raw = []
            for b in range(4):
                cl = cells[g][a][b]
                if not cl:
                    raw.append((0, b, np.zeros((BLK, BLK), np.float32)))
                for n in cl:
                    raw.append((int(cols[n]) // 4, b,
                                np.ascontiguousarray(w[n].T)))
            raw.sort(key=lambda s: s[0])
            # interleave b's: consecutive same-(a,b) slots hit the same PE
            # subarray and serialize; pick the next slot with a different b
            # from a small lookahead window (keeps t within one x-chunk)
            reordered = []
            pend = list(raw)
            prevb = None
            while pend:
                pick = 0
                for j in range(min(6, len(pend))):
                    if pend[j][1] != prevb and pend[j][0] <= pend[0][0] + 2:
                        pick = j
                        break
                slot = pend.pop(pick)
                reordered.append(slot)
                prevb = slot[1]
            raw = reordered
            first = {}
            last = {}
            for i, (_, b, _) in enumerate(raw):
                first.setdefault(b, i)
                last[b] = i
            slots = [(t, b, i == first[b], i == last[b], wt)
                     for i, (t, b, wt) in enumerate(raw)]
            lanes.append(slots)
        prog.append(lanes)

    offs, tot = [], 0
    for g in range(NG):
        offs.append(tot)
        tot += max(len(prog[g][a]) for a in range(4))
    wpk = np.zeros((128, tot * BLK), dtype=np.float32)
    for g in range(NG):
        for a in range(4):
            for idx, (_, _, _, _, wt) in enumerate(prog[g][a]):
                col = (offs[g] + idx) * BLK
                wpk[32 * a:32 * a + 32, col:col + BLK] = wt
    return prog, offs, tot, wpk, rmap


def kernel(x, w, rows, cols, out_blocks=None):
    import concourse.bass as bass
    import concourse.bacc as bacc
    import concourse.tile as tile
    import concourse.mybir as mybir
    from concourse.bass_utils import run_bass_kernel_spmd
    from contextlib import ExitStack

    x = np.asarray(x, dtype=np.float32)
    w = np.asarray(w, dtype=np.float32)
    rows = np.asarray(rows).astype(np.int64)
    cols = np.asarray(cols).astype(np.int64)

    prog, offs, tot, wpk, rmap = _build_schedule(w, rows, cols)
    offs4 = [offs[k * GC] for k in range(NWC)] + [tot]
    wpk16 = wpk.astype(BF16)

    # x^T, per-core partition-major: xarr[core, p, t*BC + j] = x[BC*core + j, 128*t + p]
    xarr = np.ascontiguousarray(
        x.reshape(NCORES, BC, NT, 128).transpose(0, 3, 2, 1)
    ).reshape(NCORES, 128, NT * BC).astype(BF16)

    f32 = mybir.dt.float32
    bf16 = mybir.dt.bfloat16
    nc = bacc.Bacc()
    xt_d = nc.declare_dram_parameter("xt", [128, NT * BC], bf16, isOutput=False)
    wp_d = nc.declare_dram_parameter("wpk", [128, tot * BLK], bf16, isOutput=False)
    yt_d = nc.declare_dram_parameter("yt", [128, NG * BC], bf16, isOutput=True)

    with tile.TileContext(nc) as tc, ExitStack() as ctx:
        xp = ctx.enter_context(tc.tile_pool(name="x", bufs=1))
        wpool = ctx.enter_context(tc.tile_pool(name="w", bufs=4))
        pp = ctx.enter_context(tc.tile_pool(name="ps", bufs=8, space="PSUM"))
        tp = ctx.enter_context(tc.tile_pool(name="tmp", bufs=3))
        yp = ctx.enter_context(tc.tile_pool(name="y", bufs=2))

        wtiles = {}

        def load_w(k):
            ncols = (offs4[k + 1] - offs4[k]) * BLK
            wsb = wpool.tile([128, ncols], bf16, tag="w", name=f"w{k}")
            nc.sync.dma_start(
                wsb[:], wp_d[:, offs4[k] * BLK:offs4[k + 1] * BLK])
            wtiles[k] = wsb

        XC = NT // XCH
        xts = [None] * XCH

        def load_x(ci, eng):
            xc = xp.tile([128, XC * BC], bf16, tag=f"xc{ci}", name=f"xc{ci}")
            eng.dma_start(
                xc[:], xt_d[:, ci * XC * BC:(ci + 1) * XC * BC])
            xts[ci] = xc

        # DMA ring is FIFO: first w chunk and first x chunk lead the queue.
        load_w(0)
        load_x(0, nc.sync)
        load_w(1)
        for ci in range(1, XCH):
            load_x(ci, nc.sync)

        def rhs_of(t):
            return xts[t // XC][:, (t % XC) * BC:(t % XC + 1) * BC]

        y4 = None
        state = {}

        def flat_of(g):
            n_g = max(len(prog[g][a]) for a in range(4))
            out = []
            for idx in range(n_g):
                for a in range(4):
                    if idx < len(prog[g][a]):
                        out.append((a, idx))
            return out

        def open_group(g):
            k = g // GC
            if g % GC == 0 and k + 2 < NWC:
                load_w(k + 2)
            state[g] = {
                "ps": [pp.tile([128, BC], f32, tag="ps", name=f"ps{a}")
                       for a in range(4)],
                "pos": 0,
                "flat": flat_of(g),
            }

        def emit_slot(g, a, idx):
            k = g // GC
            wsb = wtiles[k]
            wbase = offs[g] - offs4[k]
            t, b, st, sp, _ = prog[g][a][idx]
            wcol = (wbase + idx) * BLK
            nc.tensor.matmul(
                state[g]["ps"][a][32 * b:32 * b + 32, :],
                lhsT=wsb[32 * a:32 * a + 32, wcol:wcol + BLK],
                rhs=rhs_of(t)[32 * a:32 * a + 32, :],
                start=st, stop=sp,
                tile_position=(32 * a, 32 * b),
            )

        def fold(g):
            # PSUM read ports: ACT evacuates two banks, DVE folds two more
            # (one PSUM operand per DVE op), Pool does the SBUF-only final
            # add with the bf16 downcast.
            nonlocal y4
            ps = state[g]["ps"]
            s0 = tp.tile([128, BC], f32, tag="t0")
            nc.scalar.copy(s0[:], ps[0][:])
            s2 = tp.tile([128, BC], f32, tag="t1")
            nc.scalar.copy(s2[:], ps[2][:])
            a01 = tp.tile([128, BC], f32, tag="t2")
            nc.vector.tensor_add(a01[:], s0[:], ps[1][:])
            a23 = tp.tile([128, BC], f32, tag="t3")
            nc.vector.tensor_add(a23[:], s2[:], ps[3][:])
            if g % GC == 0:
                y4 = yp.tile([128, GC * BC], bf16, tag="y")
            nc.gpsimd.tensor_add(
                y4[:, (g % GC) * BC:(g % GC + 1) * BC], a01[:], a23[:])
            if g % GC == GC - 1:
                nc.scalar.dma_start(
                    yt_d[:, (g - GC + 1) * BC:(g + 1) * BC], y4[:])

        # Sliding 2-group window: interleave the slot streams of the two
        # oldest open groups so a stall in one (x chunk not yet resident)
        # leaves the other's eligible matmuls ahead of it in program
        # order.  Folds retire strictly in group order.
        open_group(0)
        if NG > 1:
            open_group(1)
        oldest = 0
        while oldest < NG:
            for g in (oldest, oldest + 1):
                s = state.get(g)
                if s and s["pos"] < len(s["flat"]):
                    a, idx = s["flat"][s["pos"]]
                    emit_slot(g, a, idx)
                    s["pos"] += 1
            if state[oldest]["pos"] >= len(state[oldest]["flat"]):
                fold(oldest)
                del state[oldest]
                oldest += 1
                ng = oldest + 1
                if ng < NG and ng not in state:
                    open_group(ng)

    nc.compile()

    in_maps = [{"xt": xarr[i], "wpk": wpk16} for i in range(NCORES)]
    res = run_bass_kernel_spmd(
        nc, in_maps, list(range(NCORES)),
        trace=_RUN["trace"], trace_cores=_RUN["trace_cores"],
    )
    _RUN["last"] = res

    feat = np.empty(OUT, dtype=np.int64)
    for g in range(NG):
        for b in range(4):
            feat[128 * g + 32 * b:128 * g + 32 * b + 32] = \
                32 * rmap[g, b] + np.arange(32)

    y = np.empty((B, OUT), dtype=np.float32)
    for i in range(NCORES):
        ytp = np.asarray(res.results[i]["yt"]).astype(np.float32)
        ytp = ytp.reshape(128, NG, BC).transpose(1, 0, 2).reshape(OUT, BC)
        yT = np.empty((OUT, BC), dtype=np.float32)
        yT[feat] = ytp
        y[BC * i:BC * (i + 1), :] = yT.T
    return y
